# revision 39
# baseline (speedup 1.0000x reference)
"""Trainium2 Bass kernel for nn_BaseSparseConn (gnn_message_passing).

Computes out = x @ conn + bias where conn is given in COO form
(rows = dst, cols = src of the transposed matrix):
    out.T[r, :] = sum_{e: rows[e]==r} values[e] * x[:, cols[e]]  + bias[r]

Strategy (8 NeuronCores, SPMD — one NEFF, per-core data):
  - Row-partition the output: core c owns output rows [c*12500, (c+1)*12500).
  - Per core, rows are processed in 98 blocks of 128 rows, grouped 14 blocks
    per gather group (7 groups).  A block's edges are fetched with dma_gather
    (SWDGE) from a fp16 copy of x^T laid out as (IN_F, 128); each gather
    element reads only the first 64 columns (128 B) of a 256-B-strided row
    (elem_size=64, elem_step=128 — bass's %256 elem assert only applies to
    transpose-mode gathers, so the instruction is built directly).
  - dma_gather requires int16 indices, so each block's edges are bucketed
    into 4 column ranges of 25000 and padded to a fixed chunk count.
  - Scatter-add into the 128 output rows of a block is a one-hot matmul:
    one DVE is_equal per block builds M_eq[p, kk, m] = (rows[p,kk] == m)
    against a materialized iota tile, values are multiplied into the
    gathered data in place (one DVE op per range covering the whole group),
    and the PE accumulates psum[128 rows, 64 batch] += M_eq[:,kk,:].T @
    gathered across chunks.  Gathers run on SWDGE queues 0-3.
  - Bias is a final rank-1 matmul into PSUM; the Scalar engine copies
    PSUM->SBUF and the result is DMA'd out.
"""

import numpy as np

# Problem constants (hardcoded per the harness contract)
B = 64
IN_F = 100000
OUT_F = 100000
N_CORES = 8

# Sharding / layout constants
ROWS_PER_CORE = OUT_F // N_CORES  # 12500
BLK = 128
GROUP = 14                        # blocks per gather group (98 = 7*14)
N_RANGES = 4
RANGE_W = 25000                   # unsigned int16 gather index bound
RANGE_BASE_OFF = 0                # Q7 address math is unsigned: idx >= 0 only
XPAD = 128                        # table row stride = 256 B (min DMA stride)


def _cdiv(a, b):
    return -(-a // b)


class Cfg:
    """Geometry shared between host-side data prep and the device program."""

    def __init__(self, in_f, out_f, batch, n_cores, rows_per_core, group,
                 n_ranges, range_w, cpr, xpad=128, blk=128):
        assert range_w <= 65536   # signed int16 offsets from the range mid
        self.in_f = in_f
        self.out_f = out_f
        self.batch = batch
        self.n_cores = n_cores
        self.rows_per_core = rows_per_core
        self.blk = blk
        self.group = group
        self.n_ranges = n_ranges
        self.range_w = range_w
        assert n_ranges * range_w >= in_f
        self.cpr = cpr                        # chunks per (block, range)
        self.xpad = xpad
        self.n_blocks = _cdiv(rows_per_core, blk)       # blocks per core
        assert self.n_blocks % group == 0, (self.n_blocks, group)
        self.n_groups = self.n_blocks // group
        self.cpt = n_ranges * cpr             # chunks per block
        self.slots_pg = n_ranges * group * cpr  # gather slots per group
        self.idx_w = self.slots_pg * 8        # idx free-dim per group (int16)
        self.rv_w = group * self.cpt          # rows/vals free-dim per group
        self.out_rows = self.n_blocks * blk   # padded output rows per core


def prep_host_data(cfg, x, values, bias, rows, cols):
    """Shard + lay out inputs for the device program.

    Returns per-core input dicts.
    """
    rows = np.asarray(rows).astype(np.int64)
    cols = np.asarray(cols).astype(np.int64)
    values = np.asarray(values, dtype=np.float32)
    bias = np.asarray(bias, dtype=np.float32)
    x = np.asarray(x, dtype=np.float32)

    # fp16 x^T table: row i = x[:, i] in cols 0:B, 256-B row stride
    xp = np.zeros((cfg.in_f, cfg.xpad), dtype=np.float16)
    xp[:, :cfg.batch] = x.T.astype(np.float16)

    # materialized iota along the free dim, repeated per chunk (int16: DVE
    # 16-bit compare against int16 rows, fp16 one-hot out)
    iota_rep = np.tile(np.arange(128, dtype=np.int16),
                       (128, cfg.cpt)).reshape(128, cfg.cpt * 128)

    per_core = []
    for c in range(cfg.n_cores):
        e0, e1 = np.searchsorted(rows, [c * cfg.rows_per_core,
                                        (c + 1) * cfg.rows_per_core])
        r_loc = (rows[e0:e1] - c * cfg.rows_per_core).astype(np.int64)
        col = cols[e0:e1]
        val = values[e0:e1]

        blk_id = r_loc // cfg.blk
        rng_id = col // cfg.range_w
        key = blk_id * cfg.n_ranges + rng_id
        order = np.argsort(key, kind="stable")
        key_s = key[order]
        col_s = col[order]
        val_s = val[order]
        row_s = (r_loc - blk_id * cfg.blk)[order]       # 0..127 within block

        counts = np.bincount(key_s, minlength=cfg.n_blocks * cfg.n_ranges)
        limit = cfg.cpr * 128
        assert counts.max() <= limit, (counts.max(), limit)
        starts = np.concatenate([[0], np.cumsum(counts)[:-1]])
        # position of each edge within its (block, range) bucket
        q = np.arange(len(key_s)) - starts[key_s]

        b_s = key_s // cfg.n_ranges
        r_s = key_s % cfg.n_ranges
        g_s = b_s // cfg.group
        j_s = b_s % cfg.group

        # ---- gather index array, 8x replicated across the 128 partitions.
        # One gather per (group, range): batch of group*cpr*128 indices,
        # element i lives at [i % 16, base + i // 16].
        npart_w = cfg.group * cfg.cpr * 128 // 16        # per-range free width
        # Padding slots repeat the bucket's last real index (HBM row-hit
        # instead of a cold read of row 0); value is 0 so they contribute 0.
        pad_idx = np.zeros((cfg.n_groups, cfg.n_ranges, cfg.group, cfg.cpr * 128),
                           dtype=np.int16)
        lastidx = np.zeros(cfg.n_groups * cfg.n_ranges * cfg.group,
                           dtype=np.int16)
        flatkey = (g_s * cfg.n_ranges + r_s) * cfg.group + j_s
        lastidx[flatkey] = (col_s - r_s * cfg.range_w
                            - RANGE_BASE_OFF).astype(np.int16)
        pad_idx[:] = lastidx.reshape(cfg.n_groups, cfg.n_ranges,
                                     cfg.group)[..., None]
        pad_idx = pad_idx.reshape(cfg.n_groups, cfg.n_ranges,
                                  cfg.group * cfg.cpr * 128)
        # scatter into the 16-partition wrap layout
        idx16 = np.empty((cfg.n_groups, 16, cfg.n_ranges * npart_w),
                         dtype=np.int16)
        for rr in range(cfg.n_ranges):
            blk16 = pad_idx[:, rr].reshape(cfg.n_groups, npart_w, 16)
            idx16[:, :, rr * npart_w:(rr + 1) * npart_w] = \
                blk16.transpose(0, 2, 1)
        i_in_gather = j_s * (cfg.cpr * 128) + q
        idx16[g_s, i_in_gather % 16,
              r_s * npart_w + i_in_gather // 16] = (
            col_s - r_s * cfg.range_w - RANGE_BASE_OFF).astype(np.int16)
        # Trailing negative indices would be trimmed by the Q7 generator,
        # leaving stale SBUF data in those slots (NaN risk on first use).
        # Force the final index of every (group, range) gather non-negative
        # (row RANGE_BASE_OFF of the range; its value stays 0).
        for rr in range(cfg.n_ranges):
            idx16[:, 15, (rr + 1) * npart_w - 1] = np.maximum(
                idx16[:, 15, (rr + 1) * npart_w - 1], 0)
        idx_full = np.tile(idx16, (1, 8, 1))             # replicate to 128 parts

        # ---- rows / vals arrays: (n_groups, 128, rv_w) fp16
        kk = r_s * cfg.cpr + q // 128                    # chunk id in block
        p = q % 128
        rows_a = np.zeros((cfg.n_groups, 128, cfg.rv_w), dtype=np.int16)
        vals_a = np.zeros((cfg.n_groups, 128, cfg.rv_w), dtype=np.float16)
        rows_a[g_s, p, j_s * cfg.cpt + kk] = row_s.astype(np.int16)
        vals_a[g_s, p, j_s * cfg.cpt + kk] = val_s.astype(np.float16)

        # ---- bias array: (n_groups, 1, group*blk) fp16 (rank-1 matmul row)
        bias_arr = np.zeros((cfg.n_groups, 1, cfg.group * cfg.blk),
                            dtype=np.float16)
        gg, ww = np.meshgrid(np.arange(cfg.n_groups),
                             np.arange(cfg.group * cfg.blk), indexing="ij")
        grow = c * cfg.rows_per_core + gg * cfg.group * cfg.blk + ww
        valid = grow < (c + 1) * cfg.rows_per_core
        valid &= grow < cfg.out_f
        bias_arr[gg[valid], 0, ww[valid]] = bias[grow[valid]].astype(
            np.float16)

        per_core.append({
            "xp": xp,
            "iota_rep": iota_rep,
            "idx": idx_full,
            "rowsb": rows_a,
            "valsb": vals_a,
            "biasb": bias_arr,
        })
    return per_core


def _dma_gather_thin(gp, out_ap, in_ap, idxs_ap, num_idxs, elem_size,
                     elem_step, single_packet=False, queue_num=0):
    """nc.gpsimd.dma_gather for a non-transpose DRAM->SBUF gather with a
    sub-256B element (bass's elem_size_bytes % 256 assert is a transpose-path
    restriction; decode/Q7 slice non-transpose elements by a 16 KiB packet
    size, so a 128-B element is one descriptor)."""
    import concourse.mybir as mybir
    from concourse import ap_utils
    from concourse.bass import MemorySpace, exact_div

    assert idxs_ap.dtype == mybir.dt.int16
    assert in_ap.dtype == out_ap.dtype
    assert in_ap.space == MemorySpace.DRAM
    assert idxs_ap.space == MemorySpace.SBUF
    assert out_ap.space == MemorySpace.SBUF
    assert ap_utils.ap_is_contiguous(out_ap.ap[1:])
    assert ap_utils.ap_is_contiguous(idxs_ap.ap[1:])
    assert in_ap.ap[-1][1] == out_ap.ap[-1][1] == elem_size
    assert out_ap.ap[0][1] * out_ap.ap[1][1] == num_idxs
    assert in_ap.ap[0][0] == elem_step
    stride_bytes = elem_step * mybir.dt.size(in_ap.dtype)
    stride_bytes_256 = exact_div(stride_bytes, 256)
    assert 0 < stride_bytes_256 < 256

    gp._assert_queue_num(queue_num)
    _in_ap = gp.lower_ap_dma(in_ap, for_custom_bir_dma=True)
    _idxs_ap = gp.lower_ap(idxs_ap)
    _out_ap = gp.lower_ap(out_ap)
    return gp.add_instruction(
        mybir.InstDMAGatherAnt(
            name=gp.bass.get_next_instruction_name(),
            ins=[
                *_in_ap,
                _idxs_ap,
                gp.lower_val_access(gp.to_reg(num_idxs)),
            ],
            outs=[_out_ap],
            transpose=False,
            num_idxs=num_idxs,
            elem_size=elem_size,
            stride_bytes_256=stride_bytes_256,
            gen_mode=0,
            single_packet=single_packet,
            queue_num=queue_num,
            sbuf_tokens_per_rank=0,
            sbuf_free_dim_per_rank=0,
            sbuf_free_dim_pad_per_rank=0,
            sbuf_byte_offset=0,
        )
    )


def build_program(cfg, enable_asserts=False, debug=False):
    import concourse.bacc as bacc
    import concourse.bass as bass_mod
    import concourse.mybir as mybir
    import concourse.tile as tile

    f16 = mybir.dt.float16
    f32 = mybir.dt.float32
    i16 = mybir.dt.int16

    nc = bacc.Bacc("TRN2", target_bir_lowering=False, debug=debug,
                   enable_asserts=enable_asserts, num_devices=cfg.n_cores,
                   num_swdge_queues=4)

    xp_d = nc.dram_tensor("xp", (cfg.in_f, cfg.xpad), f16, kind="ExternalInput")
    iota_d = nc.dram_tensor("iota_rep", (128, cfg.cpt * 128), i16,
                            kind="ExternalInput")
    idx_d = nc.dram_tensor("idx", (cfg.n_groups, 128,
                                   cfg.n_ranges * (cfg.group * cfg.cpr * 8)),
                           i16, kind="ExternalInput")
    rows_d = nc.dram_tensor("rowsb", (cfg.n_groups, 128, cfg.rv_w), i16,
                            kind="ExternalInput")
    vals_d = nc.dram_tensor("valsb", (cfg.n_groups, 128, cfg.rv_w), f16,
                            kind="ExternalInput")
    bias_d = nc.dram_tensor("biasb", (cfg.n_groups, 1, cfg.group * cfg.blk),
                            f16, kind="ExternalInput")
    out_d = nc.dram_tensor("out_t", (cfg.out_rows, cfg.batch), f32,
                           kind="ExternalOutput")

    npart_w = cfg.group * cfg.cpr * 8          # idx free width per range

    with tile.TileContext(nc, num_cores=cfg.n_cores) as tc:
        with (
            tc.tile_pool(name="const", bufs=1) as cp,
            tc.tile_pool(name="meta", bufs=3) as mp_meta,
            tc.tile_pool(name="gath", bufs=3) as gp,
            tc.tile_pool(name="mtile", bufs=14) as mp,
            tc.tile_pool(name="ostage", bufs=2) as op,
            tc.tile_pool(name="ps", bufs=8, space="PSUM") as pp,
        ):
            iota_t = cp.tile([128, cfg.cpt * 128], i16)
            nc.sync.dma_start(out=iota_t[:], in_=iota_d[:, :])
            ones_t = cp.tile([1, cfg.batch], f16)
            nc.vector.memset(ones_t[:], 1.0)

            # warmup: a tiny gather loads the Q7 dma_gather ucode (~6 us
            # IRAM fetch) before the first real gather needs it
            wu_idx = cp.tile([128, 8], i16)
            nc.vector.memset(wu_idx[:], 0)
            wu_out = cp.tile([128, 1, cfg.batch], f16)
            _dma_gather_thin(
                nc.gpsimd,
                out_ap=wu_out[:],
                in_ap=xp_d[0:2, 0:cfg.batch],
                idxs_ap=wu_idx[:],
                num_idxs=128,
                elem_size=cfg.batch,
                elem_step=cfg.xpad,
                single_packet=False,
                queue_num=0,
            )

            for g in range(cfg.n_groups):
                idx_t = mp_meta.tile([128, cfg.n_ranges * npart_w], i16,
                                     tag="idx")
                nc.sync.dma_start(out=idx_t[:], in_=idx_d[g])
                rows_t = mp_meta.tile([128, cfg.rv_w], i16, tag="rows")
                nc.sync.dma_start(out=rows_t[:], in_=rows_d[g])
                vals_t = mp_meta.tile([128, cfg.rv_w], f16, tag="vals")
                nc.sync.dma_start(out=vals_t[:], in_=vals_d[g])
                bias_t = mp_meta.tile([1, cfg.group * cfg.blk], f16,
                                      tag="bias")
                nc.sync.dma_start(out=bias_t[:], in_=bias_d[g])

                # M_eq builds only need rows_t — emit them BEFORE the
                # gathers and val-mults so the in-order DVE does this work
                # during the Q7 descriptor-generation window instead of
                # idling behind a gather-blocked val-mult.
                meqs = []
                for j in range(cfg.group):
                    meq = mp.tile([128, cfg.cpt, 128], f16, tag="meq")
                    rows_ap = rows_t[:, j * cfg.cpt:(j + 1) * cfg.cpt]
                    rows_bcast = rows_ap.to_broadcast([128, cfg.cpt, 128])
                    i0 = iota_t[:]
                    iota_3d = bass_mod.AP(
                        i0.tensor, i0.offset,
                        [i0.ap[0], [128, cfg.cpt], [1, 128]])
                    nc.vector.tensor_tensor(
                        out=meq[:], in0=rows_bcast, in1=iota_3d,
                        op=mybir.AluOpType.is_equal)
                    meqs.append(meq)

                gath = gp.tile([128, cfg.slots_pg, cfg.batch], f16, tag="g")
                for r in range(cfg.n_ranges):
                    base = r * cfg.range_w + RANGE_BASE_OFF
                    hi = min(base + cfg.range_w, cfg.in_f)
                    _dma_gather_thin(
                        nc.gpsimd,
                        out_ap=gath[:, r * cfg.group * cfg.cpr:
                                    (r + 1) * cfg.group * cfg.cpr, :],
                        in_ap=xp_d[base:hi, 0:cfg.batch],
                        idxs_ap=idx_t[:, r * npart_w:(r + 1) * npart_w],
                        num_idxs=cfg.group * cfg.cpr * 128,
                        elem_size=cfg.batch,
                        elem_step=cfg.xpad,
                        # one packet per descriptor: a coalesced stream of
                        # >64 descriptors/engine aborts the SDMA engine
                        single_packet=False,
                        queue_num=(g * cfg.n_ranges + r) % 4,
                    )

                # gath[p, (r,j,k), :] *= vals[p, (j,r,k)] in place — one DVE
                # op per range covering the whole group
                for r in range(cfg.n_ranges):
                    g0 = gath[:, r * cfg.group * cfg.cpr, :]
                    gsec = bass_mod.AP(
                        g0.tensor, g0.offset,
                        [g0.ap[0], [cfg.batch * cfg.cpr, cfg.group],
                         [cfg.batch, cfg.cpr], [1, cfg.batch]])
                    v0 = vals_t[:, r * cfg.cpr:r * cfg.cpr + 1]
                    vals_bcast = bass_mod.AP(
                        v0.tensor, v0.offset,
                        [v0.ap[0], [cfg.cpt, cfg.group], [1, cfg.cpr],
                         [0, cfg.batch]])
                    nc.vector.tensor_tensor(
                        out=gsec, in0=gsec, in1=vals_bcast,
                        op=mybir.AluOpType.mult)

                for j in range(cfg.group):
                    b = g * cfg.group + j
                    meq = meqs[j]
                    ps = pp.tile([128, cfg.batch], f32, tag="ps")
                    for kk in range(cfg.cpt):
                        r, k = divmod(kk, cfg.cpr)
                        slot = (r * cfg.group + j) * cfg.cpr + k
                        nc.tensor.matmul(
                            out=ps[:],
                            lhsT=meq[:, kk, :],
                            rhs=gath[:, slot, :],
                            start=(kk == 0),
                            stop=False,
                        )
                    # bias via rank-1 matmul: psum[m, :] += bias[m] * 1
                    nc.tensor.matmul(
                        out=ps[:],
                        lhsT=bias_t[0:1, j * cfg.blk:(j + 1) * cfg.blk],
                        rhs=ones_t[0:1, :],
                        start=False,
                        stop=True,
                    )
                    o_t = op.tile([128, cfg.batch], f32, tag="o")
                    nc.scalar.activation(
                        out=o_t[:], in_=ps[:],
                        func=mybir.ActivationFunctionType.Copy)
                    nc.sync.dma_start(
                        out=out_d[b * cfg.blk:(b + 1) * cfg.blk, :],
                        in_=o_t[:],
                    )

    nc.compile()
    return nc


def compute_cpr(cfg_like, rows, cols):
    """Global max chunks per (core, block, range)."""
    rows = np.asarray(rows).astype(np.int64)
    cols = np.asarray(cols).astype(np.int64)
    mx = 1
    for c in range(cfg_like["n_cores"]):
        rpc = cfg_like["rows_per_core"]
        e0, e1 = np.searchsorted(rows, [c * rpc, (c + 1) * rpc])
        r_loc = rows[e0:e1] - c * rpc
        key = (r_loc // cfg_like["blk"]) * cfg_like["n_ranges"] + \
            cols[e0:e1] // cfg_like["range_w"]
        nb = _cdiv(rpc, cfg_like["blk"])
        counts = np.bincount(key, minlength=nb * cfg_like["n_ranges"])
        mx = max(mx, int(_cdiv(int(counts.max()), 128)))
    return mx


LAST_RESULT = None  # BassKernelResults of the most recent kernel() call


def kernel(x, values, bias, rows, cols):
    global LAST_RESULT
    from concourse.bass_utils import run_bass_kernel_spmd

    rows_in = np.asarray(rows)
    cols_in = np.asarray(cols)

    cpr = compute_cpr(dict(n_cores=N_CORES, rows_per_core=ROWS_PER_CORE,
                           blk=BLK, n_ranges=N_RANGES, range_w=RANGE_W),
                      rows_in, cols_in)
    cfg = Cfg(IN_F, OUT_F, B, N_CORES, ROWS_PER_CORE, GROUP, N_RANGES,
              RANGE_W, cpr, xpad=XPAD, blk=BLK)

    per_core = prep_host_data(cfg, x, values, bias, rows_in, cols_in)
    nc = build_program(cfg)
    res = run_bass_kernel_spmd(nc, per_core, core_ids=list(range(N_CORES)))
    LAST_RESULT = res

    parts = [res.results[c]["out_t"][:ROWS_PER_CORE] for c in range(N_CORES)]
    out_t = np.concatenate(parts, axis=0)       # (OUT_F, B) f32
    return np.ascontiguousarray(out_t.T)        # (B, OUT_F) f32
